# revision 2
# baseline (speedup 1.0000x reference)
"""Cross-attention Trainium2 kernel (Bass/Tile), data-parallel over batch.

Reference computation per batch element b:
    qp = q[b] @ Wq            [S, O]
    kp = k[b] @ Wk            [S, O]
    vp = k[b] @ Wv            [S, O]
    A  = qp @ kp.T            [S, S]
    W  = softmax(A, axis=-1)  (over key axis)
    C  = W.T @ vp             [S, O]   (contract over the QUERY axis)
    out[b] = concat([q[b], C], axis=-1)

Sharding: B=8 batch elements -> 8 NeuronCores, one element per core,
weights replicated. q/k are pre-transposed on host to [D, S] so every
device matmul has its contraction dim on partitions.

Numerics: projections run as f32r matmuls (full PE rate, near-fp32);
logits and the context contraction run in fp16 with fp32 PSUM
accumulation and an exact (true row-max) softmax on the fp32 logits.
The softmax 1/Z normalization (per q row) is folded into vp rows, so
the [S,S] weight matrix is touched exactly once by the exp pass.

Everything stays resident in SBUF — no DRAM spills:
  qpT fp16 (32KB/part) + kpT fp16 (32KB) + U fp16 (64KB) + vp fp16
  (32KB) coexist only during the attention phases; queue-mode pool
  allocation lets the phase-local pools (weights, input streams) reuse
  space without LIFO constraints.
"""

import numpy as np

import concourse.bass as bass
import concourse.tile as tile
from concourse import bacc, mybir
from concourse.bass import ts
from concourse.bass_utils import run_bass_kernel_spmd

F32 = mybir.dt.float32
F32R = mybir.dt.float32r
F16 = mybir.dt.float16
BF16 = mybir.dt.bfloat16
AF = mybir.ActivationFunctionType
AX = mybir.AxisListType

P = 128  # SBUF partitions

# Full problem geometry (hardcoded: the harness calls kernel() with these)
B_FULL, S_FULL, D_FULL, O_FULL = 8, 2048, 1024, 1024
N_CORES = 8


def build_nc(S=S_FULL, D=D_FULL, O=O_FULL, mm_dt=F32R, att_dt=F16,
             p3_dt=F16, repeat=1):
    """Build + compile the per-core Bass module.

    Layouts (SBUF tiles are [partition, ...free]):
      qT, kT     DRAM [D, S]   (feature-major; host pre-transposed)
      Wq/Wk/Wv   DRAM [D, O]
      C          DRAM [S, O]
      qpT/kpT    [o_in_chunk=128, O/128 chunks, S]  att_dt  (= qp.T/kp.T)
      vp         [q_in_tile=128, S/128 tiles, O]    p3_dt
      U          [q_in_tile=128, S/128 tiles, S]    p3_dt  exp(A - rowmax)
    """
    NBS = min(512, S)   # moving free-dim block for s
    NBO = min(512, O)   # moving free-dim block for o
    DC = D // P         # contraction chunks for projections
    OC = O // P         # o chunks (partition tiles of qpT/kpT)
    QT = S // P         # q tiles
    KT = S // P         # kk tiles
    SB = S // NBS       # s blocks
    OB = O // NBO       # o blocks
    KB = S // NBS       # kk blocks inside one q-tile's logits row

    nc = bacc.Bacc("TRN2", target_bir_lowering=False, debug=False)

    qT = nc.dram_tensor("qT", [D, S], mm_dt, kind="ExternalInput").ap()
    kT = nc.dram_tensor("kT", [D, S], mm_dt, kind="ExternalInput").ap()
    wq = nc.dram_tensor("Wq", [D, O], mm_dt, kind="ExternalInput").ap()
    wk = nc.dram_tensor("Wk", [D, O], mm_dt, kind="ExternalInput").ap()
    wv = nc.dram_tensor("Wv", [D, O], mm_dt, kind="ExternalInput").ap()
    out = nc.dram_tensor("C", [S, O], F32, kind="ExternalOutput").ap()

    # feature-major views: d = chunk*128 + p
    qT_v = qT.rearrange("(c p) s -> p c s", p=P)
    kT_v = kT.rearrange("(c p) s -> p c s", p=P)
    wq_v = wq.rearrange("(c p) o -> p c o", p=P)
    wk_v = wk.rearrange("(c p) o -> p c o", p=P)
    wv_v = wv.rearrange("(c p) o -> p c o", p=P)
    out_v = out.rearrange("(t p) o -> p t o", p=P)

    with tile.TileContext(nc, pool_alloc_mode="queue") as tc:
        with (
            tc.tile_pool(name="ps", bufs=8, space="PSUM") as psum,
            tc.tile_pool(name="stats", bufs=4) as stats,
            tc.tile_pool(name="stage", bufs=4) as stage,
        ):
            for _rep in range(repeat):
                # ---------- Phase 1a: qpT = (q @ Wq).T, resident ----------
                with tc.tile_pool(name="qpt", bufs=1) as qpt_pool:
                    qpt_sb = qpt_pool.tile([P, OC, S], att_dt)
                    with (
                        tc.tile_pool(name="wq", bufs=1) as wq_pool,
                        tc.tile_pool(name="qstream", bufs=2) as qs_pool,
                    ):
                        wq_sb = wq_pool.tile([P, DC, O], mm_dt)
                        for dc in range(DC):
                            nc.sync.dma_start(
                                out=wq_sb[:, dc, :], in_=wq_v[:, dc, :]
                            )
                        for sb in range(SB):
                            q_blk = qs_pool.tile([P, DC, NBS], mm_dt, tag="qblk")
                            for dc in range(DC):
                                nc.sync.dma_start(
                                    out=q_blk[:, dc, :],
                                    in_=qT_v[:, dc, ts(sb, NBS)],
                                )
                            for oc in range(OC):
                                ps = psum.tile([P, NBS], F32, tag="ps")
                                for dc in range(DC):
                                    nc.tensor.matmul(
                                        ps,
                                        wq_sb[:, dc, ts(oc, P)],
                                        q_blk[:, dc, :],
                                        start=(dc == 0),
                                        stop=(dc == DC - 1),
                                    )
                                nc.vector.tensor_copy(
                                    out=qpt_sb[:, oc, ts(sb, NBS)], in_=ps
                                )

                    # ---------- Phase 1b: kpT resident ----------
                    with (
                        tc.tile_pool(name="vp", bufs=1) as vp_pool,
                        tc.tile_pool(name="kpt", bufs=1) as kpt_pool,
                    ):
                        vp_sb = vp_pool.tile([P, QT, O], p3_dt)
                        kpt_sb = kpt_pool.tile([P, OC, S], att_dt)

                        with (
                            tc.tile_pool(name="wk", bufs=1) as wk_pool,
                            tc.tile_pool(name="wv", bufs=1) as wv_pool,
                            tc.tile_pool(name="kstream", bufs=2) as ks_pool,
                        ):
                            wk_sb = wk_pool.tile([P, DC, O], mm_dt)
                            wv_sb = wv_pool.tile([P, DC, O], mm_dt)
                            for dc in range(DC):
                                nc.sync.dma_start(
                                    out=wk_sb[:, dc, :], in_=wk_v[:, dc, :]
                                )
                            for dc in range(DC):
                                nc.sync.dma_start(
                                    out=wv_sb[:, dc, :], in_=wv_v[:, dc, :]
                                )
                            for sb in range(SB):
                                k_blk = ks_pool.tile(
                                    [P, DC, NBS], mm_dt, tag="kblk"
                                )
                                for dc in range(DC):
                                    nc.sync.dma_start(
                                        out=k_blk[:, dc, :],
                                        in_=kT_v[:, dc, ts(sb, NBS)],
                                    )
                                for oc in range(OC):
                                    ps = psum.tile([P, NBS], F32, tag="ps")
                                    for dc in range(DC):
                                        nc.tensor.matmul(
                                            ps,
                                            wk_sb[:, dc, ts(oc, P)],
                                            k_blk[:, dc, :],
                                            start=(dc == 0),
                                            stop=(dc == DC - 1),
                                        )
                                    nc.vector.tensor_copy(
                                        out=kpt_sb[:, oc, ts(sb, NBS)], in_=ps
                                    )
                                for stl in range(NBS // P):
                                    st_i = sb * (NBS // P) + stl
                                    for ob in range(OB):
                                        ps = psum.tile([P, NBO], F32, tag="ps")
                                        for dc in range(DC):
                                            nc.tensor.matmul(
                                                ps,
                                                k_blk[:, dc, ts(stl, P)],
                                                wv_sb[:, dc, ts(ob, NBO)],
                                                start=(dc == 0),
                                                stop=(dc == DC - 1),
                                            )
                                        nc.vector.tensor_copy(
                                            out=vp_sb[:, st_i, ts(ob, NBO)],
                                            in_=ps,
                                        )

                        # ---- Phase 2: logits + softmax, U resident -------
                        with tc.tile_pool(name="u", bufs=1) as u_pool:
                            u_sb = u_pool.tile([P, QT, S], p3_dt)
                            for qt in range(QT):
                                a_ps = []
                                for kb in range(KB):
                                    ps = psum.tile([P, NBS], F32, tag="ps")
                                    for oc in range(OC):
                                        nc.tensor.matmul(
                                            ps,
                                            qpt_sb[:, oc, ts(qt, P)],
                                            kpt_sb[:, oc, ts(kb, NBS)],
                                            start=(oc == 0),
                                            stop=(oc == OC - 1),
                                        )
                                    a_ps.append(ps)
                                bmax = stats.tile([P, KB], F32, tag="bmax")
                                for kb in range(KB):
                                    nc.vector.reduce_max(
                                        out=bmax[:, kb : kb + 1],
                                        in_=a_ps[kb],
                                        axis=AX.X,
                                    )
                                negmax = stats.tile([P, 1], F32, tag="negmax")
                                nc.vector.reduce_max(
                                    out=negmax, in_=bmax, axis=AX.X, negate=True
                                )
                                zblk = stats.tile([P, KB], F32, tag="zblk")
                                for kb in range(KB):
                                    nc.scalar.activation(
                                        out=u_sb[:, qt, ts(kb, NBS)],
                                        in_=a_ps[kb],
                                        func=AF.Exp,
                                        bias=negmax,
                                        scale=1.0,
                                        accum_out=zblk[:, kb : kb + 1],
                                    )
                                z = stats.tile([P, 1], F32, tag="z")
                                nc.vector.reduce_sum(out=z, in_=zblk, axis=AX.X)
                                rz = stats.tile([P, 1], F32, tag="rz")
                                nc.vector.reciprocal(out=rz, in_=z)
                                # fold 1/Z into vp rows of this q-tile
                                nc.vector.tensor_scalar_mul(
                                    vp_sb[:, qt, :], vp_sb[:, qt, :], rz
                                )

                            # ---- Phase 3: C[kk,o] = sum_q U.T @ vp' ------
                            for kt in range(KT):
                                for ob in range(OB):
                                    ps = psum.tile([P, NBO], F32, tag="ps")
                                    for qt in range(QT):
                                        nc.tensor.matmul(
                                            ps,
                                            u_sb[:, qt, ts(kt, P)],
                                            vp_sb[:, qt, ts(ob, NBO)],
                                            start=(qt == 0),
                                            stop=(qt == QT - 1),
                                        )
                                    cst = stage.tile([P, NBO], F32, tag="cst")
                                    nc.vector.tensor_copy(out=cst, in_=ps)
                                    nc.sync.dma_start(
                                        out=out_v[:, kt, ts(ob, NBO)], in_=cst
                                    )

    nc.compile()
    return nc


# dtype of the DRAM inputs / projection matmuls (keep build + host in sync)
IN_DT = F32R

_CACHE = {}

# Set TRACE=True (e.g. from a test harness) to capture an NTFF profile;
# LAST_RESULT then holds the BassKernelResults with exec_time_ns.
TRACE = False
LAST_RESULT = None


def _get_nc():
    if "nc" not in _CACHE:
        _CACHE["nc"] = build_nc(mm_dt=IN_DT)
    return _CACHE["nc"]


def make_in_maps(np_inputs):
    """Per-core DRAM input maps from the full {q,k,Wq,Wk,Wv} dict."""
    q, k = np_inputs["q"], np_inputs["k"]
    in_dt = mybir.dt.np(IN_DT)
    wq = np.ascontiguousarray(np_inputs["Wq"], dtype=in_dt)
    wk = np.ascontiguousarray(np_inputs["Wk"], dtype=in_dt)
    wv = np.ascontiguousarray(np_inputs["Wv"], dtype=in_dt)
    in_maps = []
    for b in range(q.shape[0]):
        in_maps.append(
            {
                "qT": np.ascontiguousarray(q[b].T.astype(in_dt)),
                "kT": np.ascontiguousarray(k[b].T.astype(in_dt)),
                "Wq": wq,
                "Wk": wk,
                "Wv": wv,
            }
        )
    return in_maps


def kernel(q, k, Wq, Wk, Wv):
    """Full-input entry point: q,k [B,S,D] f32; Wq/Wk/Wv [D,O] f32.

    Returns [B, S, D+O] f32 (= concat([q, context], -1) per reference).
    """
    nc = _get_nc()
    B = q.shape[0]
    in_maps = make_in_maps({"q": q, "k": k, "Wq": Wq, "Wk": Wk, "Wv": Wv})
    global LAST_RESULT
    res = run_bass_kernel_spmd(
        nc, in_maps, core_ids=list(range(N_CORES)), trace=TRACE
    )
    LAST_RESULT = res
    ctx = np.stack([res.results[b]["C"] for b in range(B)], axis=0)
    return np.concatenate([np.asarray(q, dtype=np.float32), ctx], axis=-1)



# revision 22
# speedup vs baseline: 1.0545x; 1.0545x over previous
"""Cross-attention Trainium2 kernel (Bass/Tile), data-parallel over batch.

Reference computation per batch element b:
    qp = q[b] @ Wq            [S, O]
    kp = k[b] @ Wk            [S, O]
    vp = k[b] @ Wv            [S, O]
    A  = qp @ kp.T            [S, S]
    W  = softmax(A, axis=-1)  (over key axis)
    C  = W.T @ vp             [S, O]   (contract over the QUERY axis)
    out[b] = concat([q[b], C], axis=-1)

Sharding: B=8 batch elements -> 8 NeuronCores, one element per core,
weights replicated.

Algebraic restructure (saves one full projection):
    A = qp @ kp.T = q @ (Wq @ Wk.T) @ k.T = (q @ M) @ k.T
so per core:
    P0: M    = Wq @ Wk.T       [D, D]  f32r      (128 matmuls)
    P1: qMT  = (q @ M).T       [D, S]  ->fp16    (256 matmuls)
    P1b: vp  = k @ Wv          [S, O]  fp16      (256 matmuls, k/Wv fp16)
    P2: A    = qMT.T @ kT16    [S, S]  fp16 in, f32 psum (512 mm)
         U   = exp(A - rowmax); 1/Z folded into vp rows
    P3: CT   = vp.T-stationary x U -> C.T [O, S] (512 mm; host transposes)
Total 1664 matmuls of N=512 vs 1792 for the direct form. k arrives only
as fp16 [D, S] (resident, reused by P1b and P2); there is no f32 k copy.

Ldweights dedup: the bass compile pass pairs every 16-bit matmul with its
own InstLdweights even when consecutive matmuls share the stationary
operand. Loops order stationaries in runs (P1b: 2, P2: 4, P3: 4), and
dedup_ldweights() drops the redundant loads (verified to carry no
waits/updates) before nc.compile(): 1024 -> 384 weight loads.

DMA: few fat descriptors (>=2KB per partition line) split across the two
HWDGE queues (SP + Activation), prefetched ahead of each consuming phase.
"""

import numpy as np

import concourse.bass as bass
import concourse.tile as tile
from concourse import bacc, mybir
from concourse.bass import ts
from concourse.bass_utils import run_bass_kernel_spmd

F32 = mybir.dt.float32
F32R = mybir.dt.float32r
F16 = mybir.dt.float16
AF = mybir.ActivationFunctionType
AX = mybir.AxisListType

P = 128  # SBUF partitions

# Full problem geometry (hardcoded: the harness calls kernel() with these)
B_FULL, S_FULL, D_FULL, O_FULL = 8, 2048, 1024, 1024
N_CORES = 8

_16BIT = (mybir.dt.float16, mybir.dt.bfloat16)


def dedup_ldweights(nc):
    """Drop InstLdweights that reload the stationary operand already in the
    PE array. Runs pre-compile. A dup is dropped only when (a) its full AP
    string matches the last weight load, (b) it carries no waits/updates,
    and (c) nothing between could have changed PE weight state or the
    underlying SBUF bytes (only paired 16-bit InstMatmults intervene; any
    other PE instruction, any f32/f32r matmul -- which self-loads -- or any
    write to the stationary's memref resets the tracked state)."""
    n_dropped = 0
    for blk in nc.m.functions[0].blocks:
        insts = blk.instructions
        cur_sig = None
        cur_memref = None
        to_drop = []
        for inst in insts:
            nm = type(inst).__name__
            if nm == "InstLdweights":
                sig = (
                    str(inst.ins[0]),
                    str(inst.perf_mode),
                    str(inst.is_transpose),
                    str(inst.tile_position),
                )
                si = inst.sync_info
                clean = si is None or (not si.on_wait and not si.on_update)
                if sig == cur_sig and clean:
                    to_drop.append(inst)
                    n_dropped += 1
                else:
                    cur_sig = sig
                    cur_memref = inst.ins[0].memref
                continue
            if nm == "InstMatmult":
                wdt = inst.ins[1].dtype
                if wdt not in _16BIT:
                    cur_sig = None  # self-loading matmul clobbers the array
                continue
            eng = inst.engine
            if eng == mybir.EngineType.PE:
                cur_sig = None
                continue
            if cur_memref is not None:
                for out in inst.outs:
                    if getattr(out, "memref", None) == cur_memref:
                        cur_sig = None
                        break
        for inst in to_drop:
            insts.remove(inst)
    return n_dropped


def build_nc(S=S_FULL, D=D_FULL, O=O_FULL, repeat=1, dedup=True):
    """Build + compile the per-core Bass module.

    DRAM tensors (all per core):
      WqT, WkT   [O, D] f32r   (host pre-transposed weights)
      Wv16       [D, O] fp16
      qT         [D, S] f32r   (host pre-transposed activations)
      kT16       [D, S] fp16   (the only k copy; feeds P1b and P2)
      CT         [O, S] f32    (context, transposed; host transposes back)
    """
    NB = 512            # moving free-dim block
    DC = D // P         # contraction chunks (d or o), 8
    QT = S // P         # 128-row tiles of s, 16
    SB = S // NB        # s blocks, 4
    KB = S // NB        # key blocks per logits row, 4
    OB = O // NB        # o blocks, 2
    OT = O // P         # o tiles, 8

    nc = bacc.Bacc("TRN2", target_bir_lowering=False, debug=False)

    wqT = nc.dram_tensor("WqT", [O, D], F32R, kind="ExternalInput").ap()
    wkT = nc.dram_tensor("WkT", [O, D], F32R, kind="ExternalInput").ap()
    wv16 = nc.dram_tensor("Wv16", [D, O], F16, kind="ExternalInput").ap()
    qT = nc.dram_tensor("qT", [D, S], F32R, kind="ExternalInput").ap()
    kT16 = nc.dram_tensor("kT16", [D, S], F16, kind="ExternalInput").ap()
    out = nc.dram_tensor("CT", [O, S], F32, kind="ExternalOutput").ap()

    wqT_v = wqT.rearrange("(c p) d -> p c d", p=P)
    wkT_v = wkT.rearrange("(c p) d -> p c d", p=P)
    wv16_v = wv16.rearrange("(c p) o -> p c o", p=P)
    qT_v = qT.rearrange("(c p) s -> p c s", p=P)
    kT16_v = kT16.rearrange("(c p) s -> p c s", p=P)
    out_v = out.rearrange("(t p) s -> p t s", p=P)

    with tile.TileContext(nc, pool_alloc_mode="queue") as tc:
        with (
            tc.tile_pool(name="ps", bufs=8, space="PSUM") as psum,
            tc.tile_pool(name="stats", bufs=4) as stats,
            tc.tile_pool(name="stage", bufs=4) as stage,
        ):
            for _rep in range(repeat):
                # Persistent attention-phase tensors
                with (
                    tc.tile_pool(name="qmt", bufs=1) as qmt_pool,
                    tc.tile_pool(name="k16", bufs=1) as k16_pool,
                    tc.tile_pool(name="wv", bufs=1) as wv_pool,
                ):
                    qmt_sb = qmt_pool.tile([P, DC, S], F16)
                    k16_sb = k16_pool.tile([P, DC, S], F16)
                    wv_sb = wv_pool.tile([P, DC, O], F16)

                    # ---------- P0: M = Wq @ Wk.T (on-chip, f32r) ----------
                    with (
                        tc.tile_pool(name="m", bufs=1) as m_pool,
                        tc.tile_pool(name="qstream", bufs=2) as qs_pool,
                    ):
                        m_sb = m_pool.tile([P, DC, D], F32R)
                        q_blks = {}
                        with (
                            tc.tile_pool(name="wq", bufs=4) as wq_pool,
                            tc.tile_pool(name="wk", bufs=1) as wk_pool,
                        ):
                            wk_sb = wk_pool.tile([P, DC, D], F32R)
                            half = DC // 2
                            for h in range(2):
                                cs = slice(h * half, (h + 1) * half)
                                nc.sync.dma_start(
                                    out=wk_sb[:, cs, :], in_=wkT_v[:, cs, :]
                                )
                            # wq streams per d1t tile (stationary slices)
                            wq_ts = []
                            for d1t in range(DC):
                                wq_t = wq_pool.tile(
                                    [P, DC, P], F32R, tag="wqt",
                                    name=f"wqt{d1t}",
                                )
                                nc.scalar.dma_start(
                                    out=wq_t, in_=wqT_v[:, :, ts(d1t, P)]
                                )
                                wq_ts.append(wq_t)
                            # prefetch q blocks 0/1 behind wq on the SP
                            # queue so P1 can start the moment P0 ends
                            for sb in range(2):
                                q_blks[sb] = qs_pool.tile(
                                    [P, DC, NB], F32R, tag="qb",
                                    name=f"qb{sb}",
                                )
                                nc.sync.dma_start(
                                    out=q_blks[sb],
                                    in_=qT_v[:, :, ts(sb, NB)],
                                )
                            # k16 + wv16 land on the ACT queue behind wk,
                            # well before P1b/P2 consume them
                            nc.scalar.dma_start(out=k16_sb, in_=kT16_v)
                            nc.scalar.dma_start(out=wv_sb, in_=wv16_v)
                            # blocks 2/3 reuse ring slots; their DMAs wait
                            # on consumption of 0/1 and proceed during P1
                            for sb in range(2, SB):
                                q_blks[sb] = qs_pool.tile(
                                    [P, DC, NB], F32R, tag="qb",
                                    name=f"qb{sb}",
                                )
                                eng = nc.scalar if sb == 2 else nc.sync
                                eng.dma_start(
                                    out=q_blks[sb],
                                    in_=qT_v[:, :, ts(sb, NB)],
                                )

                            for d1t in range(DC):
                                for d2b in range(OB):
                                    ps = psum.tile([P, NB], F32, tag="ps")
                                    for oc in range(DC):
                                        nc.tensor.matmul(
                                            ps,
                                            wq_ts[d1t][:, oc, :],
                                            wk_sb[:, oc, ts(d2b, NB)],
                                            start=(oc == 0),
                                            stop=(oc == DC - 1),
                                        )
                                    nc.vector.tensor_copy(
                                        out=m_sb[:, d1t, ts(d2b, NB)], in_=ps
                                    )

                        # ------- P1: qMT = (q @ M).T -> fp16 -------
                        if True:
                            for sb in range(SB):
                                q_blk = q_blks[sb]
                                for d2c in range(DC):
                                    ps = psum.tile([P, NB], F32, tag="ps")
                                    for dc in range(DC):
                                        nc.tensor.matmul(
                                            ps,
                                            m_sb[:, dc, ts(d2c, P)],
                                            q_blk[:, dc, :],
                                            start=(dc == 0),
                                            stop=(dc == DC - 1),
                                        )
                                    nc.vector.tensor_copy(
                                        out=qmt_sb[:, d2c, ts(sb, NB)], in_=ps
                                    )

                    # ---------- P1b: vp = k @ Wv (all fp16) ----------
                    with tc.tile_pool(name="vp", bufs=1) as vp_pool:
                        vp_sb = vp_pool.tile([P, QT, O], F16)
                        for st_i in range(QT):
                            c_ps = [
                                psum.tile(
                                    [P, NB], F32, tag="ps", name=f"v_ps{ob}"
                                )
                                for ob in range(OB)
                            ]
                            # dc outer / ob inner: stationary k16 chunk
                            # repeats in runs of OB for ldweights dedup
                            for dc in range(DC):
                                for ob in range(OB):
                                    nc.tensor.matmul(
                                        c_ps[ob],
                                        k16_sb[:, dc, ts(st_i, P)],
                                        wv_sb[:, dc, ts(ob, NB)],
                                        start=(dc == 0),
                                        stop=(dc == DC - 1),
                                    )
                            for ob in range(OB):
                                nc.vector.tensor_copy(
                                    out=vp_sb[:, st_i, ts(ob, NB)],
                                    in_=c_ps[ob],
                                )

                        # ---- P2: A = qMT.T @ k16, exact softmax -> U ----
                        with tc.tile_pool(name="u", bufs=1) as u_pool:
                            u_sb = u_pool.tile([P, QT, S], F16)
                            for qt in range(QT):
                                a_ps = [
                                    psum.tile(
                                        [P, NB], F32, tag="ps",
                                        name=f"a_ps{kb}",
                                    )
                                    for kb in range(KB)
                                ]
                                # jc outer / kb inner: stationary repeats
                                # in runs of KB for ldweights dedup
                                for jc in range(DC):
                                    for kb in range(KB):
                                        nc.tensor.matmul(
                                            a_ps[kb],
                                            qmt_sb[:, jc, ts(qt, P)],
                                            k16_sb[:, jc, ts(kb, NB)],
                                            start=(jc == 0),
                                            stop=(jc == DC - 1),
                                        )
                                bmax = stats.tile([P, KB], F32, tag="bmax")
                                for kb in range(KB):
                                    nc.vector.reduce_max(
                                        out=bmax[:, kb : kb + 1],
                                        in_=a_ps[kb],
                                        axis=AX.X,
                                    )
                                negmax = stats.tile([P, 1], F32, tag="negmax")
                                nc.vector.reduce_max(
                                    out=negmax, in_=bmax, axis=AX.X, negate=True
                                )
                                zblk = stats.tile([P, KB], F32, tag="zblk")
                                for kb in range(KB):
                                    nc.scalar.activation(
                                        out=u_sb[:, qt, ts(kb, NB)],
                                        in_=a_ps[kb],
                                        func=AF.Exp,
                                        bias=negmax,
                                        scale=1.0,
                                        accum_out=zblk[:, kb : kb + 1],
                                    )
                                z = stats.tile([P, 1], F32, tag="z")
                                nc.vector.reduce_sum(out=z, in_=zblk, axis=AX.X)
                                rz = stats.tile([P, 1], F32, tag="rz")
                                nc.vector.reciprocal(out=rz, in_=z)
                                # fold 1/Z into vp rows of this q-tile
                                nc.vector.tensor_scalar_mul(
                                    vp_sb[:, qt, :], vp_sb[:, qt, :], rz
                                )

                            # ---- P3: CT[o, kk] = (U.T @ vp).T ----
                            # stationary vp[qt, ot] repeats over kb -> dedup
                            for ot in range(OT):
                                c_ps = [
                                    psum.tile(
                                        [P, NB], F32, tag="ps",
                                        name=f"c_ps{kb}",
                                    )
                                    for kb in range(KB)
                                ]
                                for qt in range(QT):
                                    for kb in range(KB):
                                        nc.tensor.matmul(
                                            c_ps[kb],
                                            vp_sb[:, qt, ts(ot, P)],
                                            u_sb[:, qt, ts(kb, NB)],
                                            start=(qt == 0),
                                            stop=(qt == QT - 1),
                                        )
                                for kb in range(KB):
                                    cst = stage.tile([P, NB], F32, tag="cst")
                                    nc.vector.tensor_copy(
                                        out=cst, in_=c_ps[kb]
                                    )
                                    eng = nc.scalar if kb % 2 else nc.sync
                                    eng.dma_start(
                                        out=out_v[:, ot, ts(kb, NB)], in_=cst
                                    )

    if dedup:
        n = dedup_ldweights(nc)
        assert n > 0, "expected redundant ldweights to drop"
    nc.compile()
    return nc


_CACHE = {}

# Set TRACE=True (e.g. from a test harness) to capture an NTFF profile;
# LAST_RESULT then holds the BassKernelResults with exec_time_ns.
TRACE = False
LAST_RESULT = None


def _get_nc():
    if "nc" not in _CACHE:
        _CACHE["nc"] = build_nc()
    return _CACHE["nc"]


def make_in_maps(np_inputs):
    """Per-core DRAM input maps from the full {q,k,Wq,Wk,Wv} dict."""
    q, k = np_inputs["q"], np_inputs["k"]
    wqT = np.ascontiguousarray(np.asarray(np_inputs["Wq"]).T, dtype=np.float32)
    wkT = np.ascontiguousarray(np.asarray(np_inputs["Wk"]).T, dtype=np.float32)
    wv16 = np.ascontiguousarray(np_inputs["Wv"], dtype=np.float16)
    in_maps = []
    for b in range(q.shape[0]):
        in_maps.append(
            {
                "WqT": wqT,
                "WkT": wkT,
                "Wv16": wv16,
                "qT": np.ascontiguousarray(
                    np.asarray(q[b]).T, dtype=np.float32
                ),
                "kT16": np.ascontiguousarray(
                    np.asarray(k[b]).T.astype(np.float16)
                ),
            }
        )
    return in_maps


def kernel(q, k, Wq, Wk, Wv):
    """Full-input entry point: q,k [B,S,D] f32; Wq/Wk/Wv [D,O] f32.

    Returns [B, S, D+O] f32 (= concat([q, context], -1) per reference).
    """
    nc = _get_nc()
    B = q.shape[0]
    in_maps = make_in_maps({"q": q, "k": k, "Wq": Wq, "Wk": Wk, "Wv": Wv})
    global LAST_RESULT
    res = run_bass_kernel_spmd(
        nc, in_maps, core_ids=list(range(N_CORES)), trace=TRACE
    )
    LAST_RESULT = res
    # CT is [O, S] per core; transpose back to [S, O]
    ctx = np.stack(
        [np.asarray(res.results[b]["CT"]).T for b in range(B)], axis=0
    )
    return np.concatenate([np.asarray(q, dtype=np.float32), ctx], axis=-1)
